# revision 11
# baseline (speedup 1.0000x reference)
"""AngleUpdate GNN message-passing kernel for 8 TRN2 NeuronCores.

Data-parallel over the angle dimension: each core processes a contiguous
slice of the 1M angles. bond_feat / atom_feat / MLP weights are replicated;
per-angle gathers run on-device via SWDGE indirect DMA.

Per-core dataflow (angle tile T = 512 = 128 partitions x G=4 groups):
  - gather bond_feat[edge_src], bond_feat[edge_dst] -> [p, j, 0:64|64:128]
  - gather atom_feat[angle_index[:,1]], DMA angle_feat slice
  - PE transposes build x^T (features on partitions, bf16 after PSUM copy)
  - mm1: h^T[128,512] = W1^T @ x^T (two 128-row chunks, accumulated)
  - relu+bias fused into the PSUM->SBUF copy
  - mm2: out[A,64] per 128-angle subtile, bias via K=1 ones-matmul
  - epilogue: angle_feat + silu(o) * sigmoid(g) -> DMA out
"""

import sys

sys.path.insert(0, "/opt/trn_rl_repo")

from contextlib import ExitStack

import numpy as np

import concourse.bass as bass
import concourse.mybir as mybir
import concourse.tile as tile
from concourse import bacc
from concourse.masks import make_identity

N_CORES = 8
N_ANGLES = 1_000_000
A_CORE = N_ANGLES // N_CORES  # 125000
N_BONDS = 500_000
N_ATOMS = 100_000
D = 64
HID = 128

G = 4  # angle groups per compute tile
T = 128 * G  # 512 angles per compute tile
TILES_PER_SUPER = 8  # gather superblock = 8 tiles = 4096 angles
SUPER = T * TILES_PER_SUPER
J = TILES_PER_SUPER * G  # gathered rows per partition per superblock

F32 = mybir.dt.float32
BF16 = mybir.dt.bfloat16
I32 = mybir.dt.int32

Relu = mybir.ActivationFunctionType.Relu
Sigmoid = mybir.ActivationFunctionType.Sigmoid
Silu = mybir.ActivationFunctionType.Silu


def n_supers_for(a_pad):
    assert a_pad % SUPER == 0
    return a_pad // SUPER


def default_a_pad():
    # smallest multiple of SUPER >= A_CORE
    return ((A_CORE + SUPER - 1) // SUPER) * SUPER  # 126976


def build_nc(a_pad=None, debug_taps=False):
    if a_pad is None:
        a_pad = default_a_pad()
    n_supers = n_supers_for(a_pad)

    nc = bacc.Bacc("TRN2", target_bir_lowering=False, debug=False)

    bond = nc.dram_tensor("bond_feat", [N_BONDS, D], F32, kind="ExternalInput").ap()
    atom = nc.dram_tensor("atom_feat", [N_ATOMS, D], F32, kind="ExternalInput").ap()
    af = nc.dram_tensor("angle_feat", [a_pad, D], F32, kind="ExternalInput").ap()
    isrc = nc.dram_tensor("idx_src", [a_pad], I32, kind="ExternalInput").ap()
    idst = nc.dram_tensor("idx_dst", [a_pad], I32, kind="ExternalInput").ap()
    ivtx = nc.dram_tensor("idx_vtx", [a_pad], I32, kind="ExternalInput").ap()
    gW1 = nc.dram_tensor("gW1", [2 * HID, HID], F32, kind="ExternalInput").ap()
    gb1 = nc.dram_tensor("gb1", [HID], F32, kind="ExternalInput").ap()
    gW2 = nc.dram_tensor("gW2", [HID, D], F32, kind="ExternalInput").ap()
    gb2 = nc.dram_tensor("gb2", [D], F32, kind="ExternalInput").ap()
    oW1 = nc.dram_tensor("oW1", [2 * HID, HID], F32, kind="ExternalInput").ap()
    ob1 = nc.dram_tensor("ob1", [HID], F32, kind="ExternalInput").ap()
    oW2 = nc.dram_tensor("oW2", [HID, D], F32, kind="ExternalInput").ap()
    ob2 = nc.dram_tensor("ob2", [D], F32, kind="ExternalInput").ap()
    out = nc.dram_tensor("out", [a_pad, D], F32, kind="ExternalOutput").ap()

    dbg = {}
    if debug_taps:
        for nm, shape, dt in [
            ("dbg_sd", [128, J * 2 * D], F32),
            ("dbg_av", [128, J * 2 * D], F32),
            ("dbg_xt01", [128, T], BF16),
            ("dbg_xt23", [128, T], BF16),
            ("dbg_hg", [128, T], BF16),
            ("dbg_ho", [128, T], BF16),
            ("dbg_gs", [128, G * D], F32),
            ("dbg_os", [128, G * D], F32),
            ("dbg_sil", [128, G * D], F32),
        ]:
            dbg[nm] = nc.dram_tensor(nm, shape, dt, kind="ExternalOutput").ap()

    with tile.TileContext(nc) as tc, ExitStack() as ctx:
        cpool = ctx.enter_context(tc.tile_pool(name="const", bufs=1))

        ident = cpool.tile([128, 128], F32)
        make_identity(nc, ident[:])

        # W1 chunks as lhsT: [K=128, chunk, M=HID], bf16 (SWDGE casts)
        gw1t = cpool.tile([128, 2, HID], BF16)
        nc.gpsimd.dma_start(out=gw1t[:], in_=gW1.rearrange("(c k) m -> k c m", k=128))
        ow1t = cpool.tile([128, 2, HID], BF16)
        nc.gpsimd.dma_start(out=ow1t[:], in_=oW1.rearrange("(c k) m -> k c m", k=128))
        gw2t = cpool.tile([128, D], BF16)
        nc.gpsimd.dma_start(out=gw2t[:], in_=gW2)
        ow2t = cpool.tile([128, D], BF16)
        nc.gpsimd.dma_start(out=ow2t[:], in_=oW2)

        gb1t = cpool.tile([128, 1], F32)
        nc.gpsimd.dma_start(out=gb1t[:], in_=gb1.rearrange("(p o) -> p o", o=1))
        ob1t = cpool.tile([128, 1], F32)
        nc.gpsimd.dma_start(out=ob1t[:], in_=ob1.rearrange("(p o) -> p o", o=1))

        # layer-2 bias, replicated G times along free dim, bf16, on partition 0
        gb2t = cpool.tile([1, G * D], BF16)
        ob2t = cpool.tile([1, G * D], BF16)
        for g in range(G):
            nc.gpsimd.dma_start(
                out=gb2t[:, g * D : (g + 1) * D],
                in_=gb2.rearrange("(o d) -> o d", o=1),
            )
            nc.gpsimd.dma_start(
                out=ob2t[:, g * D : (g + 1) * D],
                in_=ob2.rearrange("(o d) -> o d", o=1),
            )
        ones1 = cpool.tile([1, 128], BF16)
        nc.vector.memset(ones1[:], 1.0)

        idx_pool = ctx.enter_context(tc.tile_pool(name="idx", bufs=2))
        gath_pool = ctx.enter_context(tc.tile_pool(name="gath", bufs=2))
        af_pool = ctx.enter_context(tc.tile_pool(name="afp", bufs=3))
        xt_psum = ctx.enter_context(tc.tile_pool(name="xtp", bufs=3, space="PSUM"))
        xt_pool = ctx.enter_context(tc.tile_pool(name="xts", bufs=4))
        h_psum = ctx.enter_context(tc.tile_pool(name="hp", bufs=3, space="PSUM"))
        h_pool = ctx.enter_context(tc.tile_pool(name="hs", bufs=4))
        m2_psum = ctx.enter_context(tc.tile_pool(name="m2p", bufs=2, space="PSUM"))
        ep_pool = ctx.enter_context(tc.tile_pool(name="ep", bufs=3))

        for s in range(n_supers):
            sb = s * SUPER
            tis = idx_pool.tile([128, J], I32, tag="tis")
            nc.sync.dma_start(
                out=tis[:], in_=isrc[sb : sb + SUPER].rearrange("(p j) -> p j", p=128)
            )
            tid = idx_pool.tile([128, J], I32, tag="tid")
            nc.sync.dma_start(
                out=tid[:], in_=idst[sb : sb + SUPER].rearrange("(p j) -> p j", p=128)
            )
            tiv = idx_pool.tile([128, J], I32, tag="tiv")
            nc.sync.dma_start(
                out=tiv[:], in_=ivtx[sb : sb + SUPER].rearrange("(p j) -> p j", p=128)
            )

            # SWDGE indirect DMA only handles one index per partition per
            # instruction (batched layouts scramble on HW) -> J instrs/stream
            sd = gath_pool.tile([128, J, 2 * D], F32, tag="sd")
            av = gath_pool.tile([128, J, 2 * D], F32, tag="av")
            for j in range(J):
                nc.gpsimd.indirect_dma_start(
                    out=sd[:, j, 0:D],
                    out_offset=None,
                    in_=bond,
                    in_offset=bass.IndirectOffsetOnAxis(ap=tis[:, j : j + 1], axis=0),
                )
                nc.gpsimd.indirect_dma_start(
                    out=sd[:, j, D : 2 * D],
                    out_offset=None,
                    in_=bond,
                    in_offset=bass.IndirectOffsetOnAxis(ap=tid[:, j : j + 1], axis=0),
                )
                # stacked [angle_feat || vertex] tile; vtx gather fills the
                # right half, per-tile DVE copies place angle_feat left
                nc.gpsimd.indirect_dma_start(
                    out=av[:, j, D : 2 * D],
                    out_offset=None,
                    in_=atom,
                    in_offset=bass.IndirectOffsetOnAxis(ap=tiv[:, j : j + 1], axis=0),
                )

            for t in range(TILES_PER_SUPER):
                base = sb + t * T
                aft = af_pool.tile([128, G, D], F32, tag="af")
                nc.sync.dma_start(
                    out=aft[:],
                    in_=af[base : base + T, :].rearrange("(p g) d -> p g d", p=128),
                )
                nc.vector.tensor_copy(
                    out=av[:, t * G : (t + 1) * G, 0:D], in_=aft[:]
                )

                xt01p = xt_psum.tile([128, T], F32, tag="xtp")
                xt23p = xt_psum.tile([128, T], F32, tag="xtp")
                for g in range(G):
                    c = slice(g * 128, (g + 1) * 128)
                    nc.tensor.transpose(
                        out=xt01p[:, c], in_=sd[:, t * G + g, :], identity=ident[:]
                    )
                    nc.tensor.transpose(
                        out=xt23p[:, c], in_=av[:, t * G + g, :], identity=ident[:]
                    )

                xt01 = xt_pool.tile([128, T], BF16, tag="xt")
                nc.scalar.copy(out=xt01[:], in_=xt01p[:])
                xt23 = xt_pool.tile([128, T], BF16, tag="xt")
                nc.vector.tensor_copy(out=xt23[:], in_=xt23p[:])

                if debug_taps and s == 0 and t == 0:
                    nc.sync.dma_start(out=dbg["dbg_sd"], in_=sd[:].rearrange("p j c -> p (j c)"))
                    nc.sync.dma_start(out=dbg["dbg_av"], in_=av[:].rearrange("p j c -> p (j c)"))
                    nc.sync.dma_start(out=dbg["dbg_xt01"], in_=xt01[:])
                    nc.sync.dma_start(out=dbg["dbg_xt23"], in_=xt23[:])

                hgp = h_psum.tile([128, T], F32, tag="hp")
                hop = h_psum.tile([128, T], F32, tag="hp")
                nc.tensor.matmul(
                    out=hgp[:], lhsT=gw1t[:, 0, :], rhs=xt01[:], start=True, stop=False
                )
                nc.tensor.matmul(
                    out=hgp[:], lhsT=gw1t[:, 1, :], rhs=xt23[:], start=False, stop=True
                )
                nc.tensor.matmul(
                    out=hop[:], lhsT=ow1t[:, 0, :], rhs=xt01[:], start=True, stop=False
                )
                nc.tensor.matmul(
                    out=hop[:], lhsT=ow1t[:, 1, :], rhs=xt23[:], start=False, stop=True
                )

                hg = h_pool.tile([128, T], BF16, tag="hs")
                nc.scalar.activation(
                    out=hg[:], in_=hgp[:], func=Relu, bias=gb1t[:, 0:1]
                )
                ho = h_pool.tile([128, T], BF16, tag="hs")
                nc.vector.tensor_scalar(
                    out=ho[:],
                    in0=hop[:],
                    scalar1=ob1t[:, 0:1],
                    scalar2=0.0,
                    op0=mybir.AluOpType.add,
                    op1=mybir.AluOpType.max,
                )

                if debug_taps and s == 0 and t == 0:
                    nc.sync.dma_start(out=dbg["dbg_hg"], in_=hg[:])
                    nc.sync.dma_start(out=dbg["dbg_ho"], in_=ho[:])

                gp = m2_psum.tile([128, G * D], F32, tag="m2")
                op_ = m2_psum.tile([128, G * D], F32, tag="m2")
                nc.tensor.matmul(
                    out=gp[:], lhsT=ones1[:], rhs=gb2t[:], start=True, stop=False
                )
                for u in range(G):
                    nc.tensor.matmul(
                        out=gp[:, u * D : (u + 1) * D],
                        lhsT=hg[:, u * 128 : (u + 1) * 128],
                        rhs=gw2t[:],
                        start=False,
                        stop=(u == G - 1),
                    )
                nc.tensor.matmul(
                    out=op_[:], lhsT=ones1[:], rhs=ob2t[:], start=True, stop=False
                )
                for u in range(G):
                    nc.tensor.matmul(
                        out=op_[:, u * D : (u + 1) * D],
                        lhsT=ho[:, u * 128 : (u + 1) * 128],
                        rhs=ow2t[:],
                        start=False,
                        stop=(u == G - 1),
                    )

                gs = ep_pool.tile([128, G * D], F32, tag="gs")
                nc.scalar.activation(out=gs[:], in_=gp[:], func=Sigmoid)
                ob = ep_pool.tile([128, G * D], F32, tag="ob")
                nc.scalar.activation(out=ob[:], in_=op_[:], func=Sigmoid)
                # silu(o) = o * sigmoid(o); o read back from PSUM
                sil = ep_pool.tile([128, G * D], F32, tag="si")
                nc.vector.tensor_mul(out=sil[:], in0=ob[:], in1=op_[:])
                prod = ep_pool.tile([128, G * D], F32, tag="pr")
                nc.vector.tensor_mul(out=prod[:], in0=gs[:], in1=sil[:])

                if debug_taps and s == 0 and t == 0:
                    nc.sync.dma_start(out=dbg["dbg_gs"], in_=gs[:])
                    nc.sync.dma_start(out=dbg["dbg_os"], in_=ob[:])
                    nc.sync.dma_start(out=dbg["dbg_sil"], in_=sil[:])
                res = ep_pool.tile([128, G, D], F32, tag="rs")
                nc.vector.tensor_add(
                    out=res[:], in0=prod[:].rearrange("p (g d) -> p g d", g=G), in1=aft[:]
                )
                nc.sync.dma_start(
                    out=out[base : base + T, :].rearrange("(p g) d -> p g d", p=128),
                    in_=res[:],
                )

    nc.compile()
    return nc


def shard_inputs(inputs, a_pad=None):
    """Build per-core input maps from the full problem inputs."""
    if a_pad is None:
        a_pad = default_a_pad()
    n_supers = n_supers_for(a_pad)

    bond = np.ascontiguousarray(inputs["bond_feat"], dtype=np.float32)
    atom = np.ascontiguousarray(inputs["atom_feat"], dtype=np.float32)
    af = np.asarray(inputs["angle_feat"], dtype=np.float32)
    ivtx = np.asarray(inputs["angle_index"])[:, 1].astype(np.int32)
    isrc = np.asarray(inputs["edge_src"]).astype(np.int32)
    idst = np.asarray(inputs["edge_dst"]).astype(np.int32)

    common = {
        "bond_feat": bond,
        "atom_feat": atom,
        "gW1": np.ascontiguousarray(inputs["gW1"], dtype=np.float32),
        "gb1": np.ascontiguousarray(inputs["gb1"], dtype=np.float32),
        "gW2": np.ascontiguousarray(inputs["gW2"], dtype=np.float32),
        "gb2": np.ascontiguousarray(inputs["gb2"], dtype=np.float32),
        "oW1": np.ascontiguousarray(inputs["oW1"], dtype=np.float32),
        "ob1": np.ascontiguousarray(inputs["ob1"], dtype=np.float32),
        "oW2": np.ascontiguousarray(inputs["oW2"], dtype=np.float32),
        "ob2": np.ascontiguousarray(inputs["ob2"], dtype=np.float32),
    }

    def permute_idx(x_slice):
        # device expects idx[sup][p][t][g] <-> angle sup*SUPER + t*T + p*G + g
        x = np.zeros((a_pad,), np.int32)
        x[: len(x_slice)] = x_slice
        x = x.reshape(n_supers, TILES_PER_SUPER, 128, G)
        return np.ascontiguousarray(x.transpose(0, 2, 1, 3).reshape(-1))

    n_total = af.shape[0]
    per_core = (n_total + N_CORES - 1) // N_CORES
    in_maps = []
    for c in range(N_CORES):
        lo, hi = c * per_core, min((c + 1) * per_core, n_total)
        af_c = np.zeros((a_pad, D), np.float32)
        af_c[: hi - lo] = af[lo:hi]
        m = dict(common)
        m["angle_feat"] = af_c
        m["idx_src"] = permute_idx(isrc[lo:hi])
        m["idx_dst"] = permute_idx(idst[lo:hi])
        m["idx_vtx"] = permute_idx(ivtx[lo:hi])
        in_maps.append(m)
    return in_maps, per_core


_NC_CACHE = {}


def get_nc(a_pad=None):
    if a_pad is None:
        a_pad = default_a_pad()
    if a_pad not in _NC_CACHE:
        _NC_CACHE[a_pad] = build_nc(a_pad)
    return _NC_CACHE[a_pad]


def run(inputs, trace=False, **kw):
    from concourse.bass_utils import run_bass_kernel_spmd

    a_pad = default_a_pad()
    nc = get_nc(a_pad)
    in_maps, per_core = shard_inputs(inputs, a_pad)
    r = run_bass_kernel_spmd(nc, in_maps, core_ids=list(range(N_CORES)), trace=trace, **kw)
    n_total = np.asarray(inputs["angle_feat"]).shape[0]
    parts = []
    for c in range(N_CORES):
        lo, hi = c * per_core, min((c + 1) * per_core, n_total)
        parts.append(r.results[c]["out"][: hi - lo])
    return np.concatenate(parts, axis=0), r


def kernel(**inputs):
    out, _ = run(inputs, trace=False)
    return out


# revision 12
# speedup vs baseline: 1.0063x; 1.0063x over previous
"""AngleUpdate GNN message-passing kernel for 8 TRN2 NeuronCores.

Data-parallel over the angle dimension: each core processes a contiguous
slice of the 1M angles. bond_feat / atom_feat / MLP weights are replicated;
per-angle gathers run on-device via SWDGE indirect DMA.

Per-core dataflow (angle tile T = 512 = 128 partitions x G=4 groups):
  - gather bond_feat[edge_src], bond_feat[edge_dst] -> [p, j, 0:64|64:128]
  - gather atom_feat[angle_index[:,1]], DMA angle_feat slice
  - PE transposes build x^T (features on partitions, bf16 after PSUM copy)
  - mm1: h^T[128,512] = W1^T @ x^T (two 128-row chunks, accumulated)
  - relu+bias fused into the PSUM->SBUF copy
  - mm2: out[A,64] per 128-angle subtile, bias via K=1 ones-matmul
  - epilogue: angle_feat + silu(o) * sigmoid(g) -> DMA out
"""

import sys

sys.path.insert(0, "/opt/trn_rl_repo")

from contextlib import ExitStack

import numpy as np

import concourse.bass as bass
import concourse.mybir as mybir
import concourse.tile as tile
from concourse import bacc
from concourse.masks import make_identity

N_CORES = 8
N_ANGLES = 1_000_000
A_CORE = N_ANGLES // N_CORES  # 125000
N_BONDS = 500_000
N_ATOMS = 100_000
D = 64
HID = 128

G = 4  # angle groups per compute tile
T = 128 * G  # 512 angles per compute tile
TILES_PER_SUPER = 8  # gather superblock = 8 tiles = 4096 angles
SUPER = T * TILES_PER_SUPER
J = TILES_PER_SUPER * G  # gathered rows per partition per superblock

F32 = mybir.dt.float32
BF16 = mybir.dt.bfloat16
I32 = mybir.dt.int32

Relu = mybir.ActivationFunctionType.Relu
Sigmoid = mybir.ActivationFunctionType.Sigmoid
Silu = mybir.ActivationFunctionType.Silu


def n_supers_for(a_pad):
    assert a_pad % SUPER == 0
    return a_pad // SUPER


def default_a_pad():
    # smallest multiple of SUPER >= A_CORE
    return ((A_CORE + SUPER - 1) // SUPER) * SUPER  # 126976


def build_nc(a_pad=None, debug_taps=False, a_real=None):
    if a_pad is None:
        a_pad = default_a_pad()
    if a_real is None:
        a_real = a_pad
    n_supers = n_supers_for(a_pad)

    nc = bacc.Bacc("TRN2", target_bir_lowering=False, debug=False)

    bond = nc.dram_tensor("bond_feat", [N_BONDS, D], F32, kind="ExternalInput").ap()
    atom = nc.dram_tensor("atom_feat", [N_ATOMS, D], F32, kind="ExternalInput").ap()
    af = nc.dram_tensor("angle_feat", [a_pad, D], F32, kind="ExternalInput").ap()
    isrc = nc.dram_tensor("idx_src", [a_pad], I32, kind="ExternalInput").ap()
    idst = nc.dram_tensor("idx_dst", [a_pad], I32, kind="ExternalInput").ap()
    ivtx = nc.dram_tensor("idx_vtx", [a_pad], I32, kind="ExternalInput").ap()
    gW1 = nc.dram_tensor("gW1", [2 * HID, HID], F32, kind="ExternalInput").ap()
    gb1 = nc.dram_tensor("gb1", [HID], F32, kind="ExternalInput").ap()
    gW2 = nc.dram_tensor("gW2", [HID, D], F32, kind="ExternalInput").ap()
    gb2 = nc.dram_tensor("gb2", [D], F32, kind="ExternalInput").ap()
    oW1 = nc.dram_tensor("oW1", [2 * HID, HID], F32, kind="ExternalInput").ap()
    ob1 = nc.dram_tensor("ob1", [HID], F32, kind="ExternalInput").ap()
    oW2 = nc.dram_tensor("oW2", [HID, D], F32, kind="ExternalInput").ap()
    ob2 = nc.dram_tensor("ob2", [D], F32, kind="ExternalInput").ap()
    ident_d = nc.dram_tensor("ident128", [128, 128], F32, kind="ExternalInput").ap()
    out = nc.dram_tensor("out", [a_pad, D], F32, kind="ExternalOutput").ap()

    dbg = {}
    if debug_taps:
        for nm, shape, dt in [
            ("dbg_sd", [128, J * 2 * D], F32),
            ("dbg_av", [128, J * 2 * D], F32),
            ("dbg_xt01", [128, T], BF16),
            ("dbg_xt23", [128, T], BF16),
            ("dbg_hg", [128, T], BF16),
            ("dbg_ho", [128, T], BF16),
            ("dbg_gs", [128, G * D], F32),
            ("dbg_os", [128, G * D], F32),
            ("dbg_sil", [128, G * D], F32),
        ]:
            dbg[nm] = nc.dram_tensor(nm, shape, dt, kind="ExternalOutput").ap()

    with tile.TileContext(nc) as tc, ExitStack() as ctx:
        cpool = ctx.enter_context(tc.tile_pool(name="const", bufs=1))

        # keep the POOL queue free for gathers: all setup DMAs on HWDGE,
        # f32->bf16 casts on DVE/ACT
        ident = cpool.tile([128, 128], F32)
        nc.sync.dma_start(out=ident[:], in_=ident_d)

        w1f = cpool.tile([128, 2, 2, HID], F32)
        nc.sync.dma_start(out=w1f[:, :, 0, :], in_=gW1.rearrange("(c k) m -> k c m", k=128))
        nc.sync.dma_start(out=w1f[:, :, 1, :], in_=oW1.rearrange("(c k) m -> k c m", k=128))
        w1b = cpool.tile([128, 2, 2, HID], BF16)
        nc.vector.tensor_copy(out=w1b[:], in_=w1f[:])
        gw1t, ow1t = w1b[:, :, 0, :], w1b[:, :, 1, :]

        w2f = cpool.tile([128, 2, D], F32)
        nc.sync.dma_start(out=w2f[:, 0, :], in_=gW2)
        nc.sync.dma_start(out=w2f[:, 1, :], in_=oW2)
        w2b = cpool.tile([128, 2, D], BF16)
        nc.scalar.copy(out=w2b[:], in_=w2f[:])
        gw2t, ow2t = w2b[:, 0, :], w2b[:, 1, :]

        gb1t = cpool.tile([128, 1], F32)
        nc.sync.dma_start(out=gb1t[:], in_=gb1.rearrange("(p o) -> p o", o=1))
        ob1t = cpool.tile([128, 1], F32)
        nc.sync.dma_start(out=ob1t[:], in_=ob1.rearrange("(p o) -> p o", o=1))

        # layer-2 bias, replicated G times along free dim, bf16, on partition 0
        b2f = cpool.tile([1, 2, G * D], F32)
        for g in range(G):
            nc.sync.dma_start(
                out=b2f[:, 0, g * D : (g + 1) * D],
                in_=gb2.rearrange("(o d) -> o d", o=1),
            )
            nc.sync.dma_start(
                out=b2f[:, 1, g * D : (g + 1) * D],
                in_=ob2.rearrange("(o d) -> o d", o=1),
            )
        b2b = cpool.tile([1, 2, G * D], BF16)
        nc.vector.tensor_copy(out=b2b[:], in_=b2f[:])
        gb2t, ob2t = b2b[:, 0, :], b2b[:, 1, :]
        ones1 = cpool.tile([1, 128], BF16)
        nc.vector.memset(ones1[:], 1.0)

        idx_pool = ctx.enter_context(tc.tile_pool(name="idx", bufs=2))
        gath_pool = ctx.enter_context(tc.tile_pool(name="gath", bufs=2))
        af_pool = ctx.enter_context(tc.tile_pool(name="afp", bufs=3))
        xt_psum = ctx.enter_context(tc.tile_pool(name="xtp", bufs=3, space="PSUM"))
        xt_pool = ctx.enter_context(tc.tile_pool(name="xts", bufs=4))
        h_psum = ctx.enter_context(tc.tile_pool(name="hp", bufs=3, space="PSUM"))
        h_pool = ctx.enter_context(tc.tile_pool(name="hs", bufs=4))
        m2_psum = ctx.enter_context(tc.tile_pool(name="m2p", bufs=2, space="PSUM"))
        ep_pool = ctx.enter_context(tc.tile_pool(name="ep", bufs=3))

        for s in range(n_supers):
            sb = s * SUPER
            tis = idx_pool.tile([128, J], I32, tag="tis")
            nc.sync.dma_start(
                out=tis[:], in_=isrc[sb : sb + SUPER].rearrange("(p j) -> p j", p=128)
            )
            tid = idx_pool.tile([128, J], I32, tag="tid")
            nc.sync.dma_start(
                out=tid[:], in_=idst[sb : sb + SUPER].rearrange("(p j) -> p j", p=128)
            )
            tiv = idx_pool.tile([128, J], I32, tag="tiv")
            nc.sync.dma_start(
                out=tiv[:], in_=ivtx[sb : sb + SUPER].rearrange("(p j) -> p j", p=128)
            )

            # SWDGE indirect DMA only handles one index per partition per
            # instruction (batched layouts scramble on HW) -> J instrs/stream
            sd = gath_pool.tile([128, J, 2 * D], F32, tag="sd")
            av = gath_pool.tile([128, J, 2 * D], F32, tag="av")
            for j in range(J):
                if sb + (j // G) * T >= a_real:
                    continue
                nc.gpsimd.indirect_dma_start(
                    out=sd[:, j, 0:D],
                    out_offset=None,
                    in_=bond,
                    in_offset=bass.IndirectOffsetOnAxis(ap=tis[:, j : j + 1], axis=0),
                )
                nc.gpsimd.indirect_dma_start(
                    out=sd[:, j, D : 2 * D],
                    out_offset=None,
                    in_=bond,
                    in_offset=bass.IndirectOffsetOnAxis(ap=tid[:, j : j + 1], axis=0),
                )
                # stacked [angle_feat || vertex] tile; vtx gather fills the
                # right half, per-tile DVE copies place angle_feat left
                nc.gpsimd.indirect_dma_start(
                    out=av[:, j, D : 2 * D],
                    out_offset=None,
                    in_=atom,
                    in_offset=bass.IndirectOffsetOnAxis(ap=tiv[:, j : j + 1], axis=0),
                )

            for t in range(TILES_PER_SUPER):
                base = sb + t * T
                if base >= a_real:
                    continue
                aft = af_pool.tile([128, G, D], F32, tag="af")
                nc.sync.dma_start(
                    out=aft[:],
                    in_=af[base : base + T, :].rearrange("(p g) d -> p g d", p=128),
                )
                nc.vector.tensor_copy(
                    out=av[:, t * G : (t + 1) * G, 0:D], in_=aft[:]
                )

                xt01p = xt_psum.tile([128, T], F32, tag="xtp")
                xt23p = xt_psum.tile([128, T], F32, tag="xtp")
                for g in range(G):
                    c = slice(g * 128, (g + 1) * 128)
                    nc.tensor.transpose(
                        out=xt01p[:, c], in_=sd[:, t * G + g, :], identity=ident[:]
                    )
                    nc.tensor.transpose(
                        out=xt23p[:, c], in_=av[:, t * G + g, :], identity=ident[:]
                    )

                xt01 = xt_pool.tile([128, T], BF16, tag="xt")
                nc.scalar.copy(out=xt01[:], in_=xt01p[:])
                xt23 = xt_pool.tile([128, T], BF16, tag="xt")
                nc.vector.tensor_copy(out=xt23[:], in_=xt23p[:])

                if debug_taps and s == 0 and t == 0:
                    nc.sync.dma_start(out=dbg["dbg_sd"], in_=sd[:].rearrange("p j c -> p (j c)"))
                    nc.sync.dma_start(out=dbg["dbg_av"], in_=av[:].rearrange("p j c -> p (j c)"))
                    nc.sync.dma_start(out=dbg["dbg_xt01"], in_=xt01[:])
                    nc.sync.dma_start(out=dbg["dbg_xt23"], in_=xt23[:])

                hgp = h_psum.tile([128, T], F32, tag="hp")
                hop = h_psum.tile([128, T], F32, tag="hp")
                nc.tensor.matmul(
                    out=hgp[:], lhsT=gw1t[:, 0, :], rhs=xt01[:], start=True, stop=False
                )
                nc.tensor.matmul(
                    out=hgp[:], lhsT=gw1t[:, 1, :], rhs=xt23[:], start=False, stop=True
                )
                nc.tensor.matmul(
                    out=hop[:], lhsT=ow1t[:, 0, :], rhs=xt01[:], start=True, stop=False
                )
                nc.tensor.matmul(
                    out=hop[:], lhsT=ow1t[:, 1, :], rhs=xt23[:], start=False, stop=True
                )

                hg = h_pool.tile([128, T], BF16, tag="hs")
                nc.scalar.activation(
                    out=hg[:], in_=hgp[:], func=Relu, bias=gb1t[:, 0:1]
                )
                ho = h_pool.tile([128, T], BF16, tag="hs")
                nc.vector.tensor_scalar(
                    out=ho[:],
                    in0=hop[:],
                    scalar1=ob1t[:, 0:1],
                    scalar2=0.0,
                    op0=mybir.AluOpType.add,
                    op1=mybir.AluOpType.max,
                )

                if debug_taps and s == 0 and t == 0:
                    nc.sync.dma_start(out=dbg["dbg_hg"], in_=hg[:])
                    nc.sync.dma_start(out=dbg["dbg_ho"], in_=ho[:])

                gp = m2_psum.tile([128, G * D], F32, tag="m2")
                op_ = m2_psum.tile([128, G * D], F32, tag="m2")
                nc.tensor.matmul(
                    out=gp[:], lhsT=ones1[:], rhs=gb2t[:], start=True, stop=False
                )
                for u in range(G):
                    nc.tensor.matmul(
                        out=gp[:, u * D : (u + 1) * D],
                        lhsT=hg[:, u * 128 : (u + 1) * 128],
                        rhs=gw2t[:],
                        start=False,
                        stop=(u == G - 1),
                    )
                nc.tensor.matmul(
                    out=op_[:], lhsT=ones1[:], rhs=ob2t[:], start=True, stop=False
                )
                for u in range(G):
                    nc.tensor.matmul(
                        out=op_[:, u * D : (u + 1) * D],
                        lhsT=ho[:, u * 128 : (u + 1) * 128],
                        rhs=ow2t[:],
                        start=False,
                        stop=(u == G - 1),
                    )

                gs = ep_pool.tile([128, G * D], F32, tag="gs")
                nc.scalar.activation(out=gs[:], in_=gp[:], func=Sigmoid)
                ob = ep_pool.tile([128, G * D], F32, tag="ob")
                nc.scalar.activation(out=ob[:], in_=op_[:], func=Sigmoid)
                # silu(o) = o * sigmoid(o); o read back from PSUM
                sil = ep_pool.tile([128, G * D], F32, tag="si")
                nc.vector.tensor_mul(out=sil[:], in0=ob[:], in1=op_[:])
                prod = ep_pool.tile([128, G * D], F32, tag="pr")
                nc.vector.tensor_mul(out=prod[:], in0=gs[:], in1=sil[:])

                if debug_taps and s == 0 and t == 0:
                    nc.sync.dma_start(out=dbg["dbg_gs"], in_=gs[:])
                    nc.sync.dma_start(out=dbg["dbg_os"], in_=ob[:])
                    nc.sync.dma_start(out=dbg["dbg_sil"], in_=sil[:])
                res = ep_pool.tile([128, G, D], F32, tag="rs")
                nc.vector.tensor_add(
                    out=res[:], in0=prod[:].rearrange("p (g d) -> p g d", g=G), in1=aft[:]
                )
                nc.sync.dma_start(
                    out=out[base : base + T, :].rearrange("(p g) d -> p g d", p=128),
                    in_=res[:],
                )

    nc.compile()
    return nc


def shard_inputs(inputs, a_pad=None):
    """Build per-core input maps from the full problem inputs."""
    if a_pad is None:
        a_pad = default_a_pad()
    n_supers = n_supers_for(a_pad)

    bond = np.ascontiguousarray(inputs["bond_feat"], dtype=np.float32)
    atom = np.ascontiguousarray(inputs["atom_feat"], dtype=np.float32)
    af = np.asarray(inputs["angle_feat"], dtype=np.float32)
    ivtx = np.asarray(inputs["angle_index"])[:, 1].astype(np.int32)
    isrc = np.asarray(inputs["edge_src"]).astype(np.int32)
    idst = np.asarray(inputs["edge_dst"]).astype(np.int32)

    common = {
        "bond_feat": bond,
        "atom_feat": atom,
        "ident128": np.eye(128, dtype=np.float32),
        "gW1": np.ascontiguousarray(inputs["gW1"], dtype=np.float32),
        "gb1": np.ascontiguousarray(inputs["gb1"], dtype=np.float32),
        "gW2": np.ascontiguousarray(inputs["gW2"], dtype=np.float32),
        "gb2": np.ascontiguousarray(inputs["gb2"], dtype=np.float32),
        "oW1": np.ascontiguousarray(inputs["oW1"], dtype=np.float32),
        "ob1": np.ascontiguousarray(inputs["ob1"], dtype=np.float32),
        "oW2": np.ascontiguousarray(inputs["oW2"], dtype=np.float32),
        "ob2": np.ascontiguousarray(inputs["ob2"], dtype=np.float32),
    }

    def permute_idx(x_slice):
        # device expects idx[sup][p][t][g] <-> angle sup*SUPER + t*T + p*G + g
        x = np.zeros((a_pad,), np.int32)
        x[: len(x_slice)] = x_slice
        x = x.reshape(n_supers, TILES_PER_SUPER, 128, G)
        return np.ascontiguousarray(x.transpose(0, 2, 1, 3).reshape(-1))

    n_total = af.shape[0]
    per_core = (n_total + N_CORES - 1) // N_CORES
    in_maps = []
    for c in range(N_CORES):
        lo, hi = c * per_core, min((c + 1) * per_core, n_total)
        af_c = np.zeros((a_pad, D), np.float32)
        af_c[: hi - lo] = af[lo:hi]
        m = dict(common)
        m["angle_feat"] = af_c
        m["idx_src"] = permute_idx(isrc[lo:hi])
        m["idx_dst"] = permute_idx(idst[lo:hi])
        m["idx_vtx"] = permute_idx(ivtx[lo:hi])
        in_maps.append(m)
    return in_maps, per_core


_NC_CACHE = {}


def get_nc(a_pad=None):
    if a_pad is None:
        a_pad = default_a_pad()
    if a_pad not in _NC_CACHE:
        _NC_CACHE[a_pad] = build_nc(a_pad, a_real=A_CORE)
    return _NC_CACHE[a_pad]


def run(inputs, trace=False, **kw):
    from concourse.bass_utils import run_bass_kernel_spmd

    a_pad = default_a_pad()
    nc = get_nc(a_pad)
    in_maps, per_core = shard_inputs(inputs, a_pad)
    r = run_bass_kernel_spmd(nc, in_maps, core_ids=list(range(N_CORES)), trace=trace, **kw)
    n_total = np.asarray(inputs["angle_feat"]).shape[0]
    parts = []
    for c in range(N_CORES):
        lo, hi = c * per_core, min((c + 1) * per_core, n_total)
        parts.append(r.results[c]["out"][: hi - lo])
    return np.concatenate(parts, axis=0), r


def kernel(**inputs):
    out, _ = run(inputs, trace=False)
    return out


# revision 13
# speedup vs baseline: 1.0106x; 1.0043x over previous
"""AngleUpdate GNN message-passing kernel for 8 TRN2 NeuronCores.

Data-parallel over the angle dimension: each core processes a contiguous
slice of the 1M angles. bond_feat / atom_feat / MLP weights are replicated;
per-angle gathers run on-device via SWDGE indirect DMA.

Per-core dataflow (angle tile T = 512 = 128 partitions x G=4 groups):
  - gather bond_feat[edge_src], bond_feat[edge_dst], atom_feat[vertex] via
    SWDGE indirect DMA, one [128,1]-index instruction per 128 rows (the only
    batching the TRN2 qPoolDynamic ucode executes correctly; ~1.4us per
    instruction makes this the kernel's hard bottleneck)
  - DMA angle_feat slice (contiguous), DVE-stack it next to vertex rows
  - PE transposes build x^T (features on partitions, bf16 after PSUM copy)
  - mm1: h^T[128,512] = W1^T @ x^T (two 128-row K chunks, PSUM-accumulated)
  - relu+bias fused into the PSUM->SBUF copy
  - mm2: out[A,64] per 128-angle subtile, layer-2 bias via K=1 ones-matmul
  - epilogue: angle_feat + silu(o) * sigmoid(g) -> DMA out
"""

import sys

sys.path.insert(0, "/opt/trn_rl_repo")

from contextlib import ExitStack

import numpy as np

import concourse.bass as bass
import concourse.mybir as mybir
import concourse.tile as tile
from concourse import bacc

N_CORES = 8
N_ANGLES = 1_000_000
A_CORE = N_ANGLES // N_CORES  # 125000
N_BONDS = 500_000
N_ATOMS = 100_000
D = 64
HID = 128

G = 4  # angle groups per compute tile
T = 128 * G  # 512 angles per compute tile
TILES_PER_SUPER = 8  # gather superblock = 8 tiles = 4096 angles
SUPER = T * TILES_PER_SUPER
J = TILES_PER_SUPER * G  # gathered rows per partition per superblock

F32 = mybir.dt.float32
BF16 = mybir.dt.bfloat16
I32 = mybir.dt.int32

Relu = mybir.ActivationFunctionType.Relu
Sigmoid = mybir.ActivationFunctionType.Sigmoid
Silu = mybir.ActivationFunctionType.Silu


def n_supers_for(a_pad):
    assert a_pad % SUPER == 0
    return a_pad // SUPER


def default_a_pad():
    # smallest multiple of SUPER >= A_CORE
    return ((A_CORE + SUPER - 1) // SUPER) * SUPER  # 126976


def build_nc(a_pad=None, debug_taps=False, a_real=None):
    if a_pad is None:
        a_pad = default_a_pad()
    if a_real is None:
        a_real = a_pad
    n_supers = n_supers_for(a_pad)

    nc = bacc.Bacc("TRN2", target_bir_lowering=False, debug=False)

    bond = nc.dram_tensor("bond_feat", [N_BONDS, D], F32, kind="ExternalInput").ap()
    atom = nc.dram_tensor("atom_feat", [N_ATOMS, D], F32, kind="ExternalInput").ap()
    af = nc.dram_tensor("angle_feat", [a_pad, D], F32, kind="ExternalInput").ap()
    isrc = nc.dram_tensor("idx_src", [a_pad], I32, kind="ExternalInput").ap()
    idst = nc.dram_tensor("idx_dst", [a_pad], I32, kind="ExternalInput").ap()
    ivtx = nc.dram_tensor("idx_vtx", [a_pad], I32, kind="ExternalInput").ap()
    gW1 = nc.dram_tensor("gW1", [2 * HID, HID], F32, kind="ExternalInput").ap()
    gb1 = nc.dram_tensor("gb1", [HID], F32, kind="ExternalInput").ap()
    gW2 = nc.dram_tensor("gW2", [HID, D], F32, kind="ExternalInput").ap()
    gb2 = nc.dram_tensor("gb2", [D], F32, kind="ExternalInput").ap()
    oW1 = nc.dram_tensor("oW1", [2 * HID, HID], F32, kind="ExternalInput").ap()
    ob1 = nc.dram_tensor("ob1", [HID], F32, kind="ExternalInput").ap()
    oW2 = nc.dram_tensor("oW2", [HID, D], F32, kind="ExternalInput").ap()
    ob2 = nc.dram_tensor("ob2", [D], F32, kind="ExternalInput").ap()
    ident_d = nc.dram_tensor("ident128", [128, 128], F32, kind="ExternalInput").ap()
    out = nc.dram_tensor("out", [a_pad, D], F32, kind="ExternalOutput").ap()

    dbg = {}
    if debug_taps:
        for nm, shape, dt in [
            ("dbg_sd", [128, J * 2 * D], F32),
            ("dbg_av", [128, J * 2 * D], F32),
            ("dbg_xt01", [128, T], BF16),
            ("dbg_xt23", [128, T], BF16),
            ("dbg_hg", [128, T], BF16),
            ("dbg_ho", [128, T], BF16),
            ("dbg_gs", [128, G * D], F32),
            ("dbg_os", [128, G * D], F32),
            ("dbg_sil", [128, G * D], F32),
        ]:
            dbg[nm] = nc.dram_tensor(nm, shape, dt, kind="ExternalOutput").ap()

    with tile.TileContext(nc) as tc, ExitStack() as ctx:
        cpool = ctx.enter_context(tc.tile_pool(name="const", bufs=1))

        # keep the POOL queue free for gathers: all setup DMAs on HWDGE,
        # f32->bf16 casts on DVE/ACT
        ident = cpool.tile([128, 128], F32)
        nc.sync.dma_start(out=ident[:], in_=ident_d)

        w1f = cpool.tile([128, 2, 2, HID], F32)
        nc.sync.dma_start(out=w1f[:, :, 0, :], in_=gW1.rearrange("(c k) m -> k c m", k=128))
        nc.sync.dma_start(out=w1f[:, :, 1, :], in_=oW1.rearrange("(c k) m -> k c m", k=128))
        w1b = cpool.tile([128, 2, 2, HID], BF16)
        nc.vector.tensor_copy(out=w1b[:], in_=w1f[:])
        gw1t, ow1t = w1b[:, :, 0, :], w1b[:, :, 1, :]

        w2f = cpool.tile([128, 2, D], F32)
        nc.sync.dma_start(out=w2f[:, 0, :], in_=gW2)
        nc.sync.dma_start(out=w2f[:, 1, :], in_=oW2)
        w2b = cpool.tile([128, 2, D], BF16)
        nc.scalar.copy(out=w2b[:], in_=w2f[:])
        gw2t, ow2t = w2b[:, 0, :], w2b[:, 1, :]

        gb1t = cpool.tile([128, 1], F32)
        nc.sync.dma_start(out=gb1t[:], in_=gb1.rearrange("(p o) -> p o", o=1))
        ob1t = cpool.tile([128, 1], F32)
        nc.sync.dma_start(out=ob1t[:], in_=ob1.rearrange("(p o) -> p o", o=1))

        # layer-2 bias, replicated G times along free dim, bf16, on partition 0
        b2f = cpool.tile([1, 2, G * D], F32)
        for g in range(G):
            nc.sync.dma_start(
                out=b2f[:, 0, g * D : (g + 1) * D],
                in_=gb2.rearrange("(o d) -> o d", o=1),
            )
            nc.sync.dma_start(
                out=b2f[:, 1, g * D : (g + 1) * D],
                in_=ob2.rearrange("(o d) -> o d", o=1),
            )
        b2b = cpool.tile([1, 2, G * D], BF16)
        nc.vector.tensor_copy(out=b2b[:], in_=b2f[:])
        gb2t, ob2t = b2b[:, 0, :], b2b[:, 1, :]
        ones1 = cpool.tile([1, 128], BF16)
        nc.vector.memset(ones1[:], 1.0)

        idx_pool = ctx.enter_context(tc.tile_pool(name="idx", bufs=2))
        gath_pool = ctx.enter_context(tc.tile_pool(name="gath", bufs=2))
        af_pool = ctx.enter_context(tc.tile_pool(name="afp", bufs=3))
        xt_psum = ctx.enter_context(tc.tile_pool(name="xtp", bufs=3, space="PSUM"))
        xt_pool = ctx.enter_context(tc.tile_pool(name="xts", bufs=4))
        h_psum = ctx.enter_context(tc.tile_pool(name="hp", bufs=3, space="PSUM"))
        h_pool = ctx.enter_context(tc.tile_pool(name="hs", bufs=4))
        m2_psum = ctx.enter_context(tc.tile_pool(name="m2p", bufs=2, space="PSUM"))
        ep_pool = ctx.enter_context(tc.tile_pool(name="ep", bufs=3))

        for s in range(n_supers):
            sb = s * SUPER
            tis = idx_pool.tile([128, J], I32, tag="tis")
            nc.sync.dma_start(
                out=tis[:], in_=isrc[sb : sb + SUPER].rearrange("(p j) -> p j", p=128)
            )
            tid = idx_pool.tile([128, J], I32, tag="tid")
            nc.sync.dma_start(
                out=tid[:], in_=idst[sb : sb + SUPER].rearrange("(p j) -> p j", p=128)
            )
            tiv = idx_pool.tile([128, J], I32, tag="tiv")
            nc.sync.dma_start(
                out=tiv[:], in_=ivtx[sb : sb + SUPER].rearrange("(p j) -> p j", p=128)
            )

            # SWDGE indirect DMA only handles one index per partition per
            # instruction (batched layouts scramble on HW) -> J instrs/stream
            sd = gath_pool.tile([128, J, 2 * D], F32, tag="sd")
            av = gath_pool.tile([128, J, 2 * D], F32, tag="av")
            for j in range(J):
                if sb + (j // G) * T >= a_real:
                    continue
                nc.gpsimd.indirect_dma_start(
                    out=sd[:, j, 0:D],
                    out_offset=None,
                    in_=bond,
                    in_offset=bass.IndirectOffsetOnAxis(ap=tis[:, j : j + 1], axis=0),
                )
                nc.gpsimd.indirect_dma_start(
                    out=sd[:, j, D : 2 * D],
                    out_offset=None,
                    in_=bond,
                    in_offset=bass.IndirectOffsetOnAxis(ap=tid[:, j : j + 1], axis=0),
                )
                # stacked [angle_feat || vertex] tile; vtx gather fills the
                # right half, per-tile DVE copies place angle_feat left
                nc.gpsimd.indirect_dma_start(
                    out=av[:, j, D : 2 * D],
                    out_offset=None,
                    in_=atom,
                    in_offset=bass.IndirectOffsetOnAxis(ap=tiv[:, j : j + 1], axis=0),
                )

            for t in range(TILES_PER_SUPER):
                base = sb + t * T
                if base >= a_real:
                    continue
                aft = af_pool.tile([128, G, D], F32, tag="af")
                nc.sync.dma_start(
                    out=aft[:],
                    in_=af[base : base + T, :].rearrange("(p g) d -> p g d", p=128),
                )
                nc.vector.tensor_copy(
                    out=av[:, t * G : (t + 1) * G, 0:D], in_=aft[:]
                )

                xt01p = xt_psum.tile([128, T], F32, tag="xtp")
                xt23p = xt_psum.tile([128, T], F32, tag="xtp")
                for g in range(G):
                    c = slice(g * 128, (g + 1) * 128)
                    nc.tensor.transpose(
                        out=xt01p[:, c], in_=sd[:, t * G + g, :], identity=ident[:]
                    )
                    nc.tensor.transpose(
                        out=xt23p[:, c], in_=av[:, t * G + g, :], identity=ident[:]
                    )

                xt01 = xt_pool.tile([128, T], BF16, tag="xt")
                nc.scalar.copy(out=xt01[:], in_=xt01p[:])
                xt23 = xt_pool.tile([128, T], BF16, tag="xt")
                nc.vector.tensor_copy(out=xt23[:], in_=xt23p[:])

                if debug_taps and s == 0 and t == 0:
                    nc.sync.dma_start(out=dbg["dbg_sd"], in_=sd[:].rearrange("p j c -> p (j c)"))
                    nc.sync.dma_start(out=dbg["dbg_av"], in_=av[:].rearrange("p j c -> p (j c)"))
                    nc.sync.dma_start(out=dbg["dbg_xt01"], in_=xt01[:])
                    nc.sync.dma_start(out=dbg["dbg_xt23"], in_=xt23[:])

                hgp = h_psum.tile([128, T], F32, tag="hp")
                hop = h_psum.tile([128, T], F32, tag="hp")
                nc.tensor.matmul(
                    out=hgp[:], lhsT=gw1t[:, 0, :], rhs=xt01[:], start=True, stop=False
                )
                nc.tensor.matmul(
                    out=hgp[:], lhsT=gw1t[:, 1, :], rhs=xt23[:], start=False, stop=True
                )
                nc.tensor.matmul(
                    out=hop[:], lhsT=ow1t[:, 0, :], rhs=xt01[:], start=True, stop=False
                )
                nc.tensor.matmul(
                    out=hop[:], lhsT=ow1t[:, 1, :], rhs=xt23[:], start=False, stop=True
                )

                hg = h_pool.tile([128, T], BF16, tag="hs")
                nc.scalar.activation(
                    out=hg[:], in_=hgp[:], func=Relu, bias=gb1t[:, 0:1]
                )
                ho = h_pool.tile([128, T], BF16, tag="hs")
                nc.vector.tensor_scalar(
                    out=ho[:],
                    in0=hop[:],
                    scalar1=ob1t[:, 0:1],
                    scalar2=0.0,
                    op0=mybir.AluOpType.add,
                    op1=mybir.AluOpType.max,
                )

                if debug_taps and s == 0 and t == 0:
                    nc.sync.dma_start(out=dbg["dbg_hg"], in_=hg[:])
                    nc.sync.dma_start(out=dbg["dbg_ho"], in_=ho[:])

                gp = m2_psum.tile([128, G * D], F32, tag="m2")
                op_ = m2_psum.tile([128, G * D], F32, tag="m2")
                nc.tensor.matmul(
                    out=gp[:], lhsT=ones1[:], rhs=gb2t[:], start=True, stop=False
                )
                for u in range(G):
                    nc.tensor.matmul(
                        out=gp[:, u * D : (u + 1) * D],
                        lhsT=hg[:, u * 128 : (u + 1) * 128],
                        rhs=gw2t[:],
                        start=False,
                        stop=(u == G - 1),
                    )
                nc.tensor.matmul(
                    out=op_[:], lhsT=ones1[:], rhs=ob2t[:], start=True, stop=False
                )
                for u in range(G):
                    nc.tensor.matmul(
                        out=op_[:, u * D : (u + 1) * D],
                        lhsT=ho[:, u * 128 : (u + 1) * 128],
                        rhs=ow2t[:],
                        start=False,
                        stop=(u == G - 1),
                    )

                gs = ep_pool.tile([128, G * D], F32, tag="gs")
                nc.scalar.activation(out=gs[:], in_=gp[:], func=Sigmoid)
                ob = ep_pool.tile([128, G * D], F32, tag="ob")
                nc.scalar.activation(out=ob[:], in_=op_[:], func=Sigmoid)
                # silu(o) = o * sigmoid(o); o read back from PSUM
                sil = ep_pool.tile([128, G * D], F32, tag="si")
                nc.vector.tensor_mul(out=sil[:], in0=ob[:], in1=op_[:])
                prod = ep_pool.tile([128, G * D], F32, tag="pr")
                nc.vector.tensor_mul(out=prod[:], in0=gs[:], in1=sil[:])

                if debug_taps and s == 0 and t == 0:
                    nc.sync.dma_start(out=dbg["dbg_gs"], in_=gs[:])
                    nc.sync.dma_start(out=dbg["dbg_os"], in_=ob[:])
                    nc.sync.dma_start(out=dbg["dbg_sil"], in_=sil[:])
                res = ep_pool.tile([128, G, D], F32, tag="rs")
                nc.vector.tensor_add(
                    out=res[:], in0=prod[:].rearrange("p (g d) -> p g d", g=G), in1=aft[:]
                )
                nc.sync.dma_start(
                    out=out[base : base + T, :].rearrange("(p g) d -> p g d", p=128),
                    in_=res[:],
                )

    nc.compile()
    return nc


def shard_inputs(inputs, a_pad=None):
    """Build per-core input maps from the full problem inputs."""
    if a_pad is None:
        a_pad = default_a_pad()
    n_supers = n_supers_for(a_pad)

    bond = np.ascontiguousarray(inputs["bond_feat"], dtype=np.float32)
    atom = np.ascontiguousarray(inputs["atom_feat"], dtype=np.float32)
    af = np.asarray(inputs["angle_feat"], dtype=np.float32)
    ivtx = np.asarray(inputs["angle_index"])[:, 1].astype(np.int32)
    isrc = np.asarray(inputs["edge_src"]).astype(np.int32)
    idst = np.asarray(inputs["edge_dst"]).astype(np.int32)

    common = {
        "bond_feat": bond,
        "atom_feat": atom,
        "ident128": np.eye(128, dtype=np.float32),
        "gW1": np.ascontiguousarray(inputs["gW1"], dtype=np.float32),
        "gb1": np.ascontiguousarray(inputs["gb1"], dtype=np.float32),
        "gW2": np.ascontiguousarray(inputs["gW2"], dtype=np.float32),
        "gb2": np.ascontiguousarray(inputs["gb2"], dtype=np.float32),
        "oW1": np.ascontiguousarray(inputs["oW1"], dtype=np.float32),
        "ob1": np.ascontiguousarray(inputs["ob1"], dtype=np.float32),
        "oW2": np.ascontiguousarray(inputs["oW2"], dtype=np.float32),
        "ob2": np.ascontiguousarray(inputs["ob2"], dtype=np.float32),
    }

    def permute_idx(x_slice):
        # device expects idx[sup][p][t][g] <-> angle sup*SUPER + t*T + p*G + g
        x = np.zeros((a_pad,), np.int32)
        x[: len(x_slice)] = x_slice
        x = x.reshape(n_supers, TILES_PER_SUPER, 128, G)
        return np.ascontiguousarray(x.transpose(0, 2, 1, 3).reshape(-1))

    n_total = af.shape[0]
    per_core = (n_total + N_CORES - 1) // N_CORES
    in_maps = []
    for c in range(N_CORES):
        lo, hi = c * per_core, min((c + 1) * per_core, n_total)
        af_c = np.zeros((a_pad, D), np.float32)
        af_c[: hi - lo] = af[lo:hi]
        m = dict(common)
        m["angle_feat"] = af_c
        m["idx_src"] = permute_idx(isrc[lo:hi])
        m["idx_dst"] = permute_idx(idst[lo:hi])
        m["idx_vtx"] = permute_idx(ivtx[lo:hi])
        in_maps.append(m)
    return in_maps, per_core


_NC_CACHE = {}


def get_nc(a_pad=None):
    if a_pad is None:
        a_pad = default_a_pad()
    if a_pad not in _NC_CACHE:
        _NC_CACHE[a_pad] = build_nc(a_pad, a_real=A_CORE)
    return _NC_CACHE[a_pad]


def run(inputs, trace=False, **kw):
    from concourse.bass_utils import run_bass_kernel_spmd

    a_pad = default_a_pad()
    nc = get_nc(a_pad)
    in_maps, per_core = shard_inputs(inputs, a_pad)
    r = run_bass_kernel_spmd(nc, in_maps, core_ids=list(range(N_CORES)), trace=trace, **kw)
    n_total = np.asarray(inputs["angle_feat"]).shape[0]
    parts = []
    for c in range(N_CORES):
        lo, hi = c * per_core, min((c + 1) * per_core, n_total)
        parts.append(r.results[c]["out"][: hi - lo])
    return np.concatenate(parts, axis=0), r


def kernel(**inputs):
    out, _ = run(inputs, trace=False)
    return out
